# revision 47
# baseline (speedup 1.0000x reference)
"""GQA attention (RoPE + softmax + o_proj) on 8 Trainium2 NeuronCores.

Problem shapes (hardcoded): hidden_states [4, 2048, 2048], 16 q heads,
4 kv heads, head_dim 128, rope cos/sin tables given as inputs.

Sharding: core c -> (batch b = c // 2, q-head half = c % 2).  Each core
computes 8 q heads + their 2 kv heads for one batch and produces a
partial o_proj output [2048, 2048]; the host sums the two halves per
batch (tensor parallel, no device collectives).

All matmuls run in fp16 (1 cycle/row on PE) with fp32 PSUM accumulation:
  - v/k projections are kt-major: one accumulation group per output tile
    stays open across the 16 hid tiles, so the PE starts on the first
    ~128KB of hs instead of waiting for the whole 4MB block
  - RoPE via a DVE partition shuffle (head dim host-permuted so +-64
    pairs sit 16 apart; sign lives in the pre-negated sin table),
    software-pipelined one projection behind the PE
  - scores^T[t, s] with k^T tiles stationary; exp via ScalarE (fused
    1/sqrt(d) scale) reads two PSUM banks per instruction and writes
    P^T fp16 straight to SBUF; the exp-dependent stages of iteration i
    are emitted after the score matmuls of iteration i+1 (software
    pipeline) so PE never idles waiting for ScalarE
  - softmax denominators: three fp16 DVE pairwise-add levels over P^T,
    then one all-ones stationary matmul (result replicated across
    partitions = pre-broadcast), fast DVE reciprocal, fused
    normalize+cast on the attn PSUM->SBUF copyback
  - attn^T[d, s] = v-tiles stationary @ P^T; o_proj with attn^T tiles
    stationary over Wo, split into [128, 512] chunks that are
    interleaved into the NEXT block's head iterations: they feed the PE
    while ScalarE drains exp, so the attention region runs at the PE
    roofline instead of the exp roofline.  PSUM copybacks ride on DVE
    (ScalarE is exp-saturated) and output DMAs (fp16) alternate queues.
  - si=3's q projections are deferred out of phase A into the first
    attention block's iterations (the only region with no o_proj chunks
    to fill the exp-pacing slack); their weights are re-fetched per
    head into small double-buffered tiles.

Scheduling refinements (558us -> ~537us):
  - post() emits the 16 attn matmuls FIRST (gated only on exp + psum
    rotation), then the denominator tree, so the PE's fill stream never
    waits on DVE; the tree is split into two halves so only 3 adds + the
    ones-matmul remain after a block's last exp
  - startup input DMAs fan out over all three DMA rings (sync/scalar/
    gpsimd) with the first hs chunk split across two rings, wk on
    gpsimd parallel to wv on sync, and q head 0/1 weights racing the
    first hs block so the q loop starts the moment v/k finish
  - h==0 iterations flush reserve o_proj chunks BEFORE post() so their
    psum allocations don't gate on the not-yet-finished softmax chain
  - the final si=3 o_proj flush alternates copybacks between ScalarE
    (exp-idle by then) and DVE and fans DMAs over three rings
  - psum pools at psB=3/psc=2/psq=1 bufs: the deeper psB rotation keeps
    o_proj chunk allocations from gating on the previous block's
    normalize at si boundaries
  Fragile invariants (measured, do not "fix"): mid-kernel o_proj
  copybacks must stay on DVE (a single ScalarE copy in the exp stream
  costs +47us); gpsimd partition_all_reduce is 3.5us/call (slower than
  the ones-matmul).  Beware: the device sometimes enters a ~0.84x-clock
  state where everything measures ~19% slower (640us-class runs with
  unchanged gap structure) -- re-measure before judging a change.
"""

import sys

import numpy as np

B, S, HID = 4, 2048, 2048
NH, NKV, HD = 16, 4, 128
NH_L = 8        # q heads per core
NKV_L = 2       # kv heads per core
GROUP = NH // NKV
P = 128
ST = 512        # s-block (matmul free dim)
NSB = S // ST   # 4 s-blocks
KT = HID // P   # 16 contraction tiles over hidden
TT = S // P     # 16 key/t tiles
SCALE = 1.0 / float(np.sqrt(HD))

_CACHE = {}


def _build():
    if "/opt/trn_rl_repo" not in sys.path:
        sys.path.insert(0, "/opt/trn_rl_repo")
    import concourse.mybir as mybir
    from concourse import bacc
    from concourse.tile import TileContext
    from concourse.tile_rust import add_dep_helper

    dt = mybir.dt
    f16, f32 = dt.float16, dt.float32

    nc = bacc.Bacc("TRN2", target_bir_lowering=False, debug=False, num_devices=8)
    # host-pretiled layouts (see kernel() below)
    hsT = nc.dram_tensor("hsT", [P, NSB, KT, ST], f16, kind="ExternalInput").ap()
    wq = nc.dram_tensor("wq", [P, NH_L, KT, HD], f16, kind="ExternalInput").ap()
    wk = nc.dram_tensor("wk", [P, KT, NKV_L * HD], f16, kind="ExternalInput").ap()
    wv = nc.dram_tensor("wv", [P, KT, NKV_L * HD], f16, kind="ExternalInput").ap()
    wo = nc.dram_tensor("wo", [P, NH_L, HID], f16, kind="ExternalInput").ap()
    cosT = nc.dram_tensor("cosT", [HD, S], f16, kind="ExternalInput").ap()
    sinT = nc.dram_tensor("sinT", [HD, S], f16, kind="ExternalInput").ap()
    out = nc.dram_tensor("out", [S, HID], f16, kind="ExternalOutput").ap()

    EXP = mybir.ActivationFunctionType.Exp

    with TileContext(nc) as tc:
        with (
            tc.tile_pool(name="consts", bufs=1) as consts,
            tc.tile_pool(name="qkv", bufs=1) as qkvp,
            tc.tile_pool(name="trig", bufs=1) as trig,
            tc.tile_pool(name="ropes", bufs=2) as smalls,
            tc.tile_pool(name="hs3", bufs=1) as hs3p,
        ):
            ones = consts.tile([P, P], f16, tag="ones")
            nc.vector.memset(ones, 1.0)
            # rotate_half as an intra-quadrant partition shuffle (the head
            # dim is host-permuted so +-64 pairs sit 16 apart per quadrant;
            # the sign lives in the pre-negated sin table)
            SHUF = list(range(16, 32)) + list(range(0, 16))

            q_sb = qkvp.tile([P, NH_L, S], f16, tag="q")
            k_sb = qkvp.tile([P, NKV_L, S], f16, tag="k")
            v_sb = qkvp.tile([P, TT, NKV_L * HD], f16, tag="v")
            # trig tables, rope scratch, and the si=3 hs block live at outer
            # scope: si=3's q projections are deferred into phase B's first
            # attention block (whose PE would otherwise idle behind exp)
            cos_sb = trig.tile([HD, S], f16, tag="cos")
            sin_sb = trig.tile([HD, S], f16, tag="sin")
            hs3f = hs3p.tile([P, KT, ST], f16, tag="hs3")

            pending = []

            def rope_flush():
                qc, s0, dst, dsti = pending.pop(0)
                rc = smalls.tile([P, ST], f16, tag="rc")
                nc.vector.stream_shuffle(rc, qc, SHUF)
                t1 = smalls.tile([P, ST], f16, tag="t1")
                nc.vector.tensor_mul(t1, qc, cos_sb[:, s0 : s0 + ST])
                t2 = smalls.tile([P, ST], f16, tag="t2")
                nc.vector.tensor_mul(t2, rc, sin_sb[:, s0 : s0 + ST])
                nc.vector.tensor_add(dst[:, dsti, s0 : s0 + ST], t1, t2)

            # ---------------- Phase A: projections + RoPE ----------------
            with (
                tc.tile_pool(name="wqkv", bufs=1) as wp,
                tc.tile_pool(name="hs", bufs=2) as hsp,
                tc.tile_pool(name="psvk", bufs=1, space="PSUM") as psvk,
                tc.tile_pool(name="psA", bufs=2, space="PSUM") as psA,
            ):
                # first hs block + wv + wk arrive in fine-grained chunks
                # spread across all three DMA rings (sync/scalar/gpsimd):
                # the kt-major v/k groups start on chunk 0
                hs_blks = {}
                hs_first = hsp.tile([P, KT, ST], f16, tag="hs")
                wv_sb = wp.tile([P, KT, NKV_L * HD], f16, tag="wv")
                wk_sb = wp.tile([P, KT, NKV_L * HD], f16, tag="wk")
                wq_sb = wp.tile([P, NH_L, KT, HD], f16, tag="wq")
                nc.scalar.dma_start(
                    out=hs_first[:, 0:1, 0:256], in_=hsT[:, 0, 0:1, 0:256]
                )
                nc.sync.dma_start(out=wv_sb[:, 0:2, :], in_=wv[:, 0:2, :])
                nc.gpsimd.dma_start(out=wk_sb[:, 0:2, :], in_=wk[:, 0:2, :])
                nc.sync.dma_start(
                    out=hs_first[:, 0:1, 256:512], in_=hsT[:, 0, 0:1, 256:512]
                )
                nc.scalar.dma_start(out=hs_first[:, 1:3, :], in_=hsT[:, 0, 1:3, :])
                nc.sync.dma_start(out=wv_sb[:, 2:6, :], in_=wv[:, 2:6, :])
                nc.gpsimd.dma_start(out=wk_sb[:, 2:6, :], in_=wk[:, 2:6, :])
                nc.scalar.dma_start(out=hs_first[:, 3:6, :], in_=hsT[:, 0, 3:6, :])
                nc.sync.dma_start(out=wv_sb[:, 6:16, :], in_=wv[:, 6:16, :])
                nc.gpsimd.dma_start(out=wk_sb[:, 6:16, :], in_=wk[:, 6:16, :])
                nc.scalar.dma_start(out=hs_first[:, 6:11, :], in_=hsT[:, 0, 6:11, :])
                # last hs chunk rides gpsimd (done with wk by then); sync
                # would sit behind all of wv
                hs0_dma = nc.gpsimd.dma_start(
                    out=hs_first[:, 11:16, :], in_=hsT[:, 0, 11:16, :]
                )
                # q head 0/1 weights right behind the hs block on both rings
                # so the q projections start right as the v/k kt-loop ends
                nc.sync.dma_start(out=wq_sb[:, 0, :, :], in_=wq[:, 0, :, :])
                nc.scalar.dma_start(out=wq_sb[:, 1, :, :], in_=wq[:, 1, :, :])
                hs_blks[0] = hs_first
                hs_dmas = [hs0_dma]

                # trig tables on the scalar ring tail: not needed until the
                # first rope_flush (~26us), and gpsimd must start hs1-3
                nc.scalar.dma_start(out=cos_sb, in_=cosT)
                nc.scalar.dma_start(out=sin_sb, in_=sinT)

                for h in range(2, NH_L):  # per-head DMAs so early heads land first
                    nc.sync.dma_start(out=wq_sb[:, h, :, :], in_=wq[:, h, :, :])

                for si in range(NSB):
                    s0 = si * ST
                    if si in hs_blks:
                        hs_blk = hs_blks[si]
                    else:
                        hs_blk = hs3f if si == NSB - 1 else hsp.tile(
                            [P, KT, ST], f16, tag="hs"
                        )
                        # gpsimd queue is otherwise idle, so chaining these
                        # issues behind the previous block stalls nothing
                        hd = nc.gpsimd.dma_start(out=hs_blk, in_=hsT[:, si, :, :])
                        add_dep_helper(
                            hd.ins,
                            hs_dmas[-1].ins,
                            sync=True,
                            reason="stagger hs blocks",
                        )
                        hs_dmas.append(hd)

                    # kt-major v + k: 6 accumulation groups stay open across
                    # the 16 hid tiles (4 banks), consuming hs chunks on
                    # arrival
                    # one PSUM bank per open accumulation group (pv groups
                    # padded to a full 2KB bank)
                    pv_t = psvk.tile([P, NSB, ST], f32, tag="pv")
                    pk_t = psvk.tile([P, NKV_L, ST], f32, tag="pk")
                    for kt in range(KT):
                        # v,v,k,v,v,k: the 213ns k multiplies give the
                        # weight loader slack to stay ahead of the 107ns
                        # v multiplies (LDWEIGHTS is ~97ns per stationary)
                        for sj, j in [(0, None), (1, None), (None, 0),
                                      (2, None), (3, None), (None, 1)]:
                            if sj is not None:
                                nc.tensor.matmul(
                                    pv_t[:, sj, 0 : NKV_L * HD],
                                    lhsT=hs_blk[:, kt, sj * P : (sj + 1) * P],
                                    rhs=wv_sb[:, kt, :],
                                    start=(kt == 0),
                                    stop=(kt == KT - 1),
                                )
                            else:
                                nc.tensor.matmul(
                                    pk_t[:, j, :],
                                    lhsT=wk_sb[:, kt, j * HD : (j + 1) * HD],
                                    rhs=hs_blk[:, kt, :],
                                    start=(kt == 0),
                                    stop=(kt == KT - 1),
                                )
                    for sj in range(NSB):
                        if si == NSB - 1:
                            # DVE copy: ScalarE must be free to start phase
                            # B's first exp right at the A->B boundary
                            nc.vector.tensor_copy(
                                v_sb[:, si * NSB + sj, :],
                                pv_t[:, sj, 0 : NKV_L * HD],
                            )
                        else:
                            nc.scalar.copy(
                                v_sb[:, si * NSB + sj, :],
                                pv_t[:, sj, 0 : NKV_L * HD],
                            )
                    for j in range(NKV_L):
                        kc = smalls.tile([P, ST], f16, tag="qc")
                        nc.vector.tensor_copy(kc, pk_t[:, j, :])
                        pending.append((kc, s0, k_sb, j))
                        while pending:
                            rope_flush()

                    if si == NSB - 1:
                        continue  # si=3 q projections run inside phase B
                    for h in range(NH_L):
                        pm = psA.tile([P, ST], f32, tag="ps")
                        for kt in range(KT):
                            nc.tensor.matmul(
                                pm,
                                lhsT=wq_sb[:, h, kt, :],
                                rhs=hs_blk[:, kt, :],
                                start=(kt == 0),
                                stop=(kt == KT - 1),
                            )
                        qc = smalls.tile([P, ST], f16, tag="qc")
                        nc.vector.tensor_copy(qc, pm)
                        pending.append((qc, s0, q_sb, h))
                        while pending:
                            rope_flush()

            # ---------------- Phase B: attention + interleaved o_proj ------
            with (
                tc.tile_pool(name="wo", bufs=1) as wop,
                tc.tile_pool(name="attn", bufs=1) as ap_,
                tc.tile_pool(name="pblk", bufs=2) as pp,
                tc.tile_pool(name="phalf", bufs=1) as php,
                tc.tile_pool(name="rcps", bufs=2) as rcpp,
                tc.tile_pool(name="outp", bufs=8) as op_,
                tc.tile_pool(name="wqf", bufs=2) as wqfp,
                tc.tile_pool(name="psB", bufs=3, space="PSUM") as psB,
                tc.tile_pool(name="psc", bufs=2, space="PSUM") as pscp,
                tc.tile_pool(name="psq", bufs=1, space="PSUM") as psqp,
            ):
                wo_sb = wop.tile([P, NH_L, HID], f16, tag="wo")
                wod = nc.sync.dma_start(out=wo_sb, in_=wo)
                add_dep_helper(
                    wod.ins, hs0_dma.ins, sync=True, reason="defer wo behind hs0"
                )
                attnT = ap_.tile([P, NH_L, S], f16, tag="attnT")
                QT = TT // 4
                oq = [nc.sync, nc.gpsimd, nc.sync, nc.gpsimd]
                tailq = [nc.sync, nc.gpsimd, nc.scalar]

                # si=3's q projections, deferred from phase A: one head per
                # si=0 iteration, feeding the PE while ScalarE drains exp
                wqf_tiles = {}

                def qfill_dma(h):
                    if h < NH_L:
                        wt = wqfp.tile([P, KT, HD], f16, tag="wqf")
                        nc.sync.dma_start(out=wt, in_=wq[:, h, :, :])
                        wqf_tiles[h] = wt

                def qfill(h):
                    wt = wqf_tiles.pop(h)
                    pm = psqp.tile([P, ST], f32, tag="psq")
                    for kt in range(KT):
                        nc.tensor.matmul(
                            pm,
                            lhsT=wt[:, kt, :],
                            rhs=hs3f[:, kt, :],
                            start=(kt == 0),
                            stop=(kt == KT - 1),
                        )
                    qc = smalls.tile([P, ST], f16, tag="qc")
                    nc.vector.tensor_copy(qc, pm)
                    pending.append((qc, (NSB - 1) * ST, q_sb, h))
                    while pending:
                        rope_flush()

                def score_pair(h, si, pblk, t2_):
                    j = h // GROUP
                    s0 = si * ST
                    psc = pscp.tile([P, 2, ST], f32, tag="psc")
                    for u in range(2):
                        tt = 2 * t2_ + u
                        nc.tensor.matmul(
                            psc[:, u, :],
                            lhsT=k_sb[:, j, tt * P : (tt + 1) * P],
                            rhs=q_sb[:, h, s0 : s0 + ST],
                            start=True,
                            stop=True,
                        )
                    nc.scalar.activation(
                        out=pblk[:, 2 * t2_ : 2 * t2_ + 2, :],
                        in_=psc,
                        func=EXP,
                        scale=SCALE,
                    )

                def scores(h, si, pblk):
                    for t2_ in range(TT // 2):
                        score_pair(h, si, pblk, t2_)

                def post(h, si, pblk):
                    j = h // GROUP
                    s0 = si * ST
                    # attn matmuls FIRST: they gate only on exp + psB, so
                    # they fill the PE while the DVE tree runs behind them
                    pat = psB.tile([P, ST], f32, tag="ps")
                    for tt in range(TT):
                        nc.tensor.matmul(
                            pat,
                            lhsT=v_sb[:, tt, j * HD : (j + 1) * HD],
                            rhs=pblk[:, tt, :],
                            start=(tt == 0),
                            stop=(tt == TT - 1),
                        )
                    # denominator tree in two halves: the first half gates
                    # only on the block's first 4 exp pairs, so after the
                    # last exp just 3 adds remain before the ones-matmul
                    ph = php.tile([P, TT // 2, ST], f16, tag="ph")
                    nc.vector.tensor_add(
                        ph[:, 0:4, :], pblk[:, 0:4, :], pblk[:, 4:8, :]
                    )
                    nc.vector.tensor_add(
                        ph[:, 0:2, :], ph[:, 0:2, :], ph[:, 2:4, :]
                    )
                    nc.vector.tensor_add(ph[:, 0, :], ph[:, 0, :], ph[:, 1, :])
                    nc.vector.tensor_add(
                        ph[:, 4:8, :], pblk[:, 8:12, :], pblk[:, 12:16, :]
                    )
                    nc.vector.tensor_add(
                        ph[:, 4:6, :], ph[:, 4:6, :], ph[:, 6:8, :]
                    )
                    nc.vector.tensor_add(ph[:, 4, :], ph[:, 4, :], ph[:, 5, :])
                    nc.vector.tensor_add(ph[:, 0, :], ph[:, 0, :], ph[:, 4, :])
                    pcs = psB.tile([P, ST], f32, tag="ps")
                    nc.tensor.matmul(
                        pcs, lhsT=ones, rhs=ph[:, 0, :], start=True, stop=True
                    )
                    rcp = rcpp.tile([P, ST], f32, tag="rcp")
                    nc.vector.reciprocal_approx_fast(out=rcp, in_=pcs)
                    nc.vector.tensor_mul(attnT[:, h, s0 : s0 + ST], pat, rcp)

                def o_chunk(st, ni, tail=False):
                    # one [128, 512] o_proj chunk; PSUM copyback on DVE (ACT
                    # is saturated by exp) and the out DMA alternates queues.
                    # In the tail (all exps drained) copybacks alternate
                    # DVE/ACT and the DMAs fan out over four rings.
                    po = psB.tile([P, ST], f32, tag="ps")
                    for ft in range(NH_L):
                        nc.tensor.matmul(
                            po,
                            lhsT=attnT[:, ft, st * P : (st + 1) * P],
                            rhs=wo_sb[:, ft, ni * ST : (ni + 1) * ST],
                            start=(ft == 0),
                            stop=(ft == NH_L - 1),
                        )
                    oc = op_.tile([P, ST], f16, tag="oc")
                    k = st * 4 + ni
                    if tail and k % 2 == 0:
                        nc.scalar.copy(oc, po)
                    else:
                        nc.vector.tensor_copy(oc, po)
                    qs = tailq if tail else oq
                    qs[k % len(qs)].dma_start(
                        out=out[st * P : (st + 1) * P, ni * ST : (ni + 1) * ST],
                        in_=oc,
                    )

                # o_proj chunks of block si-1 are interleaved into block si's
                # head iterations: they fill the PE while ScalarE drains exp,
                # so the attention region runs at the PE roofline instead of
                # the exp roofline.
                prev = None
                pending_o = []
                qfill_dma(0)
                qfill_dma(1)
                for si in range(NSB):
                    for h in range(NH_L):
                        if si == 0:
                            qfill(h)
                            qfill_dma(h + 2)
                        pblk = pp.tile([P, TT, ST], f16, tag="pblk")
                        scores(h, si, pblk)
                        if h == 0:
                            if prev is not None:
                                # fill from OLDER blocks' reserve BEFORE
                                # post: o_chunk psum allocs must not gate on
                                # this block's softmax chain
                                for _ in range(2):
                                    if len(pending_o) > 16:
                                        o_chunk(*pending_o.pop(0))
                                post(*prev)  # block si-1 fully posted
                                pending_o += [
                                    ((si - 1) * (ST // P) + sj, ni)
                                    for sj in range(ST // P)
                                    for ni in range(HID // ST)
                                ]
                        else:
                            # keep 2 chunks in reserve to fill the PE while
                            # the last head's exp drains before the tail
                            n = 2 if len(pending_o) > 4 or si < NSB - 1 else 1
                            for _ in range(n):
                                if pending_o:
                                    o_chunk(*pending_o.pop(0))
                            post(*prev)
                        prev = (h, si, pblk)
                for st, ni in pending_o:
                    o_chunk(st, ni)
                post(*prev)
                pending_o = [
                    ((NSB - 1) * (ST // P) + sj, ni)
                    for sj in range(ST // P)
                    for ni in range(HID // ST)
                ]
                for st, ni in pending_o:
                    o_chunk(st, ni, tail=True)

    nc.compile()
    return nc


def _get_nc():
    if "nc" not in _CACHE:
        _CACHE["nc"] = _build()
    return _CACHE["nc"]


def kernel(hidden_states, cos, sin, Wq, Wk, Wv, Wo):
    if "/opt/trn_rl_repo" not in sys.path:
        sys.path.insert(0, "/opt/trn_rl_repo")
    from concourse.bass_utils import run_bass_kernel_spmd

    hidden_states = np.asarray(hidden_states, dtype=np.float32)
    cos = np.asarray(cos, dtype=np.float32)
    sin = np.asarray(sin, dtype=np.float32)
    Wq = np.asarray(Wq, dtype=np.float32)
    Wk = np.asarray(Wk, dtype=np.float32)
    Wv = np.asarray(Wv, dtype=np.float32)
    Wo = np.asarray(Wo, dtype=np.float32)

    nc = _get_nc()
    dperm = np.concatenate(
        [np.r_[16 * q : 16 * q + 16, 64 + 16 * q : 64 + 16 * q + 16] for q in range(4)]
    )
    dsign = np.where(np.arange(HD) % 32 < 16, -1.0, 1.0).astype(np.float32)

    # pretiled host layouts: partition index first, contiguous per DMA slice
    def tile_khid(w):  # [HID, F] -> [P, KT, F]
        return np.ascontiguousarray(
            w.reshape(KT, P, w.shape[1]).transpose(1, 0, 2)
        ).astype(np.float16)

    in_maps = []
    hsT_b = [
        np.ascontiguousarray(
            hidden_states[b].T.reshape(KT, P, NSB, ST).transpose(1, 2, 0, 3)
        ).astype(np.float16)
        for b in range(B)
    ]
    cosT_b = [np.ascontiguousarray(cos[b].T[dperm]).astype(np.float16) for b in range(B)]
    sinT_b = [
        np.ascontiguousarray(sin[b].T[dperm] * dsign[:, None]).astype(np.float16)
        for b in range(B)
    ]
    for c in range(2 * B):
        b, half = c // 2, c % 2
        fq = slice(half * NH_L * HD, (half + 1) * NH_L * HD)
        fkv = slice(half * NKV_L * HD, (half + 1) * NKV_L * HD)
        wq_t = tile_khid(Wq[:, fq]).reshape(P, KT, NH_L, HD).transpose(0, 2, 1, 3)
        wq_t = wq_t[:, :, :, dperm]
        wk_t = tile_khid(Wk[:, fkv]).reshape(P, KT, NKV_L, HD)
        wk_t = wk_t[:, :, :, dperm].reshape(P, KT, NKV_L * HD)
        wo_t = np.ascontiguousarray(
            Wo[fq, :].reshape(NH_L, P, HID).transpose(1, 0, 2)
        ).astype(np.float16)
        in_maps.append(
            {
                "hsT": hsT_b[b],
                "wq": np.ascontiguousarray(wq_t),
                "wk": np.ascontiguousarray(wk_t),
                "wv": tile_khid(Wv[:, fkv]),
                "wo": wo_t,
                "cosT": cosT_b[b],
                "sinT": sinT_b[b],
            }
        )

    res = run_bass_kernel_spmd(nc, in_maps, list(range(2 * B)))
    _CACHE["last_results"] = res

    out = np.empty((B, S, HID), dtype=np.float32)
    for b in range(B):
        out[b] = res.results[2 * b]["out"].astype(np.float32) + res.results[
            2 * b + 1
        ]["out"].astype(np.float32)
    return out

